# revision 50
# baseline (speedup 1.0000x reference)
"""Trainium2 Bass kernel for nn_AttentionMergeMask (8 NeuronCores, SPMD).

Reference computation (per sample b):
    K[c,k]   = (fg[c,k]+EPS) / ||fg[:,k]+EPS|| * m[k]        (k = pixel idx, 1024)
    att[c,p] = sum_k K[c,k] * A[k,p]                          (A = attention_scores[b])
    final    = att*(1-m) + fg*m
    out      = comb_w @ [fg; final] + comb_b

Sharding: pure data parallel — batch 32 split 4 samples per core across the
8 cores; small combiner weights replicated.

Per-core kernel (Tile framework):
  - work in a transposed [pixel, channel] layout so every norm/mask step is a
    per-partition scalar op:
      PE-transpose fg -> FT[pix,ch] (PSUM); ACT Square+accum_out reads the
      PSUM directly for normsq[pix]; s = 32*m*sqrt(1/normsq) (the x32 keeps
      the fp8 K quantization out of e4m3's denormal range)
  - mm1 in fp8 e4m3 with the DoubleRow perf mode (2 contraction rows per
    PE pass, 0.5 cycles/row): the big attention matmul runs 2x faster than
    fp16. Accuracy is restored by residual ("split") quantization:
      A is host-prescaled by the blend coefficient (1-m_p)*32 and shipped as
      A8 + Ar8 (e4m3 value + e4m3 residual, same bytes as fp16); K^T is
      quantized on-device as K8 + Kr8 (DVE + Pool writes).
      mm1 accumulates A8@K8 (4 DoubleRow instrs) plus the cross terms
      Ar8@K8 + A8@Kr8 (8 DoubleRow instrs, pairing the two matrices per
      instruction); only the doubly-small Ar8@Kr8 term is dropped.
  - blend is ONE DVE scalar_tensor_tensor per block: since A carries
    (1-m)*32 and K carries x32, psum = 1024*(1-m)*att, so
    fin' = ft*(1024*m) + psum = 1024*final; the 1/1024 descale rides the
    ACT Copy evacuation of the back-transpose for free.
  - PE-transpose fin' back to natural; mm2 (fp16) out = W^T @ [fg; final]
    with host-pretransposed weights; comb_b added during PSUM evacuation.

DMA/overlap structure (v2): fg for ALL samples is shipped as its own tensor
and lands in the first ~7us (sample 0's slice split off so its prep chain
starts immediately); the attention stream (A8+Ar8, 16KB/partition/sample)
then streams behind it. All four samples' prep chains (transposes, norms,
K-quant) complete DURING the ~29us A stream instead of trailing it, and the
mm1/fin stages interleave so fin(b-1) fills the PE idle window while A(b)
lands. Engine rebalance: ft-evac and the Kr8 residual quantization move to
the otherwise-idle Pool engine, cutting DVE busy so the tail is PE-limited
only. Cost-model (TimelineSim) per-core time dropped ~72us -> ~50us.

I/O layouts are host-pre-permuted partition-major so every DMA transfer is
one large contiguous run per partition (128 descriptors): per-exec
descriptor count stays ~1.2k (the axon PJRT path charges ~50ns/descriptor).

Numerics: EPS dropped (relative impact ~1e-7 for unit-scale randn inputs).
Measured on-hw output error vs the fp32 reference: ~7e-4 norm-relative
(TRN_MM1=dr2 default), allclose(2e-2) True. TRN_MM1=dr1 additionally drops
Ar8 (halves attn DMA bytes, mm1 in 8 instrs) at 1.33e-2 error but fails the
elementwise allclose(2e-2); TRN_MM1=f16 is the fp16 path (~4.2e-4, no fp8).
"""

import os
import numpy as np

NCORES = 8
BS, CH, H, W = 32, 256, 32, 32
HW = H * W                     # 1024
SPC = BS // NCORES             # samples per core = 4
NJ = HW // 128                 # 8 pixel chunks
NCB = CH // 128                # 2 channel blocks
NIC = (2 * CH) // 128          # 4 cat chunks

MM_F32R = os.environ.get("TRN_MM_F32R", "1") == "1"      # mm1 (A @ K^T)
MM2_F32R = os.environ.get("TRN_MM2_F32R", "1") == "1"    # mm2 (W @ cat)
TIN_F32R = os.environ.get("TRN_TIN_F32R", "0") == "1"    # fg transposes
NEWTON = os.environ.get("TRN_NEWTON", "0") == "1"
DT16 = os.environ.get("TRN_DT16", "1") == "1"    # fp16 data path (A, fg, W, K^T)
# mm1 mode: "dr2" = fp8 DoubleRow with full residual correction (A=A8+Ar8,
# K=K8+Kr8, dropping only the tiny Ar8@Kr8 cross term; fp16-level accuracy),
# "dr1" = fp8 DoubleRow, A quantized to a single e4m3 level (K keeps its
# residual); halves A DMA traffic at ~1.3e-2 rel err. "f16" = original path.
MM1 = os.environ.get("TRN_MM1", "dr2")
SQ_ENG = os.environ.get("TRN_SQ", "act")         # norm-square engine
SQS = os.environ.get("TRN_SQS", "mix")           # norm-square source: psum|sbuf|mix
FTE = os.environ.get("TRN_FTE", "dve")           # ft-evac engine: pool|dve|act
KRP = os.environ.get("TRN_KRP", "mixsub")        # Kr8 residual: mixsub|dve
OB0 = os.environ.get("TRN_OB0", "dve")           # mm2 ob0 evac: dve|act|pool
SCHED = os.environ.get("TRN_SCHED", "searched")  # stage order
PSB = os.environ.get("TRN_PSB", "332")           # psum bufs: pstr,psa,ps2
PEW = int(os.environ.get("TRN_PEW", "8"))        # PE p-state warm matmuls
FEV = os.environ.get("TRN_FEV", "act")           # finaln evac: act|pool
# mm2 W1@fg half: "prep" precomputes it during the prep phase (hidden under
# the A DMA stream; fin only does W2@final and merges via one stt per block),
# "fin" computes the full contraction in fin_stage (legacy).
W1P = os.environ.get("TRN_W1P", "prep")
# NOTE: GPSIMD/Pool cannot read PSUM on TRN2 (BIR verifier rejects it), so
# every PSUM evacuation must be on DVE or ACT; Pool only gets SBUF->SBUF ops.
W1E = os.environ.get("TRN_W1E", "act")           # W1@fg evac engine: act|dve

_cache = {}


def _build():
    import concourse.bass as bass
    import concourse.tile as tile
    import concourse.mybir as mybir
    from concourse import bacc
    from concourse.bass import ts

    f32 = mybir.dt.float32
    f32r = mybir.dt.float32r
    f16 = mybir.dt.float16
    f8 = mybir.dt.float8e4
    DR = mybir.MatmulPerfMode.DoubleRow
    if DT16:
        d_mm1 = d_mm2 = d_tin = f16
        d_ft = f16      # FT / scratch / blend tiles
    else:
        d_mm1 = f32r if MM_F32R else f32    # ah, kt
        d_mm2 = f32r if MM2_F32R else f32   # wtt, fgn(mm2 rhs), finaln
        d_tin = f32r if TIN_F32R else f32   # fgn/ident/pst transpose path
        d_ft = f32
    AF = mybir.ActivationFunctionType
    ALU = mybir.AluOpType

    nc = bacc.Bacc(
        "TRN2",
        target_bir_lowering=False,
        debug=False,
        enable_asserts=False,
    )
    # All inputs are host-pre-permuted to partition-major layouts so every
    # DMA moves one large contiguous run per partition (128 descriptors per
    # transfer instead of 1024): the axon path costs ~50ns per descriptor
    # per execution.
    if MM1 == "dr2":
        # innermost per k-row: [Ar8 | A8] so cross-pass lhsT pairs are
        # contiguous; layout [b, p, kchunk(8)*which*pix]
        ABODY = 8 * 2 * HW
        at_d = nc.dram_tensor("attn", [SPC, 128, ABODY], f8,
                              kind="ExternalInput")
    elif MM1 == "dr1":
        ABODY = 8 * HW
        at_d = nc.dram_tensor("attn", [SPC, 128, ABODY], f8,
                              kind="ExternalInput")
    else:
        ABODY = 8 * HW
        at_d = nc.dram_tensor("attn", [SPC, 128, ABODY * 2], mybir.dt.uint8,
                              kind="ExternalInput")
    # fg for all samples, partition-major: [p, b, cblock, pix]
    fg_d = nc.dram_tensor("fgd", [128, SPC - 1, NCB, HW], d_tin,
                          kind="ExternalInput")
    # merged constants: [ identity(128) | comb_w^T blocks (NIC*CH) |
    #                    comb_b bit-packed as f16 pairs (2*NCB) | mask (SPC*NJ
    #                    packed as f32 bit-pairs -> 2*SPC*NJ f16 cols) ]
    NCONST = 128 + NIC * CH + 2 * NCB + 2 * SPC * NJ
    # sample 0's fg rides behind the constants so the whole early block is
    # ONE DMA (128 descriptors) and the A stream starts a transfer earlier
    NCMFG = NCONST + NCB * HW * (1 if DT16 else 2)
    cm_d = nc.dram_tensor("cm", [128, NCMFG], f16, kind="ExternalInput")
    # out layout [p, b, pixhalf, ochunk, 512]: batched end-of-kernel DMAs with
    # a large contiguous run per partition
    out_d = nc.dram_tensor("out", [128, SPC, 2, NCB, 512],
                           f16 if DT16 else f32, kind="ExternalOutput")

    with tile.TileContext(nc) as tc:
        with (
            tc.tile_pool(name="const", bufs=1) as cpool,
            tc.tile_pool(name="sb", bufs=4 if DT16 else 2) as pool,
            tc.tile_pool(name="abuf", bufs=4) as apool,
            tc.tile_pool(name="pst", bufs=int(PSB[0]),
                         space=bass.MemorySpace.PSUM) as pstr,
            tc.tile_pool(name="psa", bufs=int(PSB[1]),
                         space=bass.MemorySpace.PSUM) as psatt,
            tc.tile_pool(name="ps2", bufs=int(PSB[2]),
                         space=bass.MemorySpace.PSUM) as psmm2,
        ):
            cmt = cpool.tile([128, NCMFG], f16)
            ident = cmt[:, 0:128]
            cst = cmt[:, 128:128 + NIC * CH]
            wtt = cst.rearrange("p (ic o) -> p ic o", ic=NIC)
            _o = 128 + NIC * CH
            b2t = cmt[:, _o:_o + 2 * NCB].bitcast(f32)
            _o += 2 * NCB
            m_all = cmt[:, _o:_o + 2 * SPC * NJ].bitcast(f32).rearrange(
                "p (b j) -> p b j", b=SPC
            )
            fg0sb = cmt[:, NCONST:].bitcast(d_tin).rearrange(
                "p (c f) -> p c f", c=NCB
            )
            fgsb = cpool.tile([128, SPC - 1, NCB, HW], d_tin)

            def fgv(b):
                return fg0sb if b == 0 else fgsb[:, b - 1]
            outsb_all = cpool.tile([128, SPC, 2, NCB, 512], d_ft)
            # Pre-warm the ACT spline-table sets (Square, Sqrt) at t=0 so the
            # ~2.6us table loads stay off the first sample's norm chain.
            warm = cpool.tile([128, 1], f32)
            nc.gpsimd.memset(warm[:], 1.0)
            nc.scalar.activation(warm[:], warm[:], AF.Square)
            nc.scalar.activation(warm[:], warm[:], AF.Sqrt)
            if PEW:
                # PE p-state warm-up: the tensor engine clocks up only after
                # sustained execution (0.65 -> 1.2 -> 2.4 GHz); burn dummy
                # matmuls during the initial DMA wait so sample 0's
                # transposes and first mm1 groups run at full clock.
                pwd = cpool.tile([128, 256], d_tin)
                nc.gpsimd.memset(pwd[:], 1.0)
                pwps = psatt.tile([128, CH], f32, tag="psa", name="pwps")
                for _ in range(PEW):
                    nc.tensor.matmul(
                        pwps[:], pwd[:, 0:128], pwd[:],
                        start=True, stop=True,
                    )
            state = {}

            def prep_A(b):
                # one DMA per sample covering A (128 descriptors); sample 0
                # splits into pixel halves so mm1(0) starts on the first half
                # ~3us before the full transfer lands. Layout is jhalf-outer:
                # [p, jhalf, kchunk, (which,) 512pix].
                if MM1 == "f16":
                    raw = apool.tile([128, ABODY * 2], mybir.dt.uint8,
                                     tag="A")
                    a = raw[:].bitcast(d_mm1).rearrange(
                        "p (h k f) -> p h k f", h=2, k=8
                    )
                else:
                    raw = apool.tile([128, ABODY], f8, tag="A")
                    if MM1 == "dr2":
                        a = raw[:].rearrange(
                            "p (h k two f) -> p h k two f", h=2, k=8, two=2
                        )
                    else:
                        a = raw[:].rearrange(
                            "p (h k f) -> p h k f", h=2, k=8
                        )
                if b == 0 and os.environ.get("TRN_ASPL", "0") == "1":
                    nbytes = raw.shape[1] // 2
                    nc.sync.dma_start(raw[:, :nbytes], at_d[b, :, :nbytes])
                    nc.sync.dma_start(raw[:, nbytes:], at_d[b, :, nbytes:])
                else:
                    nc.sync.dma_start(raw[:], at_d[b][:])
                state[("A", b)] = a

            def prep(b):
                fgn = fgv(b)
                m_til = m_all[:, b, :]

                # ---- transpose fg -> FT[pix, ch], normsq via Square+accum ----
                ft = pool.tile([128, NJ, CH], d_ft, tag="ft")
                nsq = pool.tile([128, NJ], f32, tag="nsq")
                rin = pool.tile([128, NJ], f32, tag="rin")
                rsq = pool.tile([128, NJ], f32, tag="rsq")
                s_til = pool.tile([128, NJ], f32, tag="stil")
                for jp in range(NJ // 2):
                    pst = pstr.tile([128, 2 * CH], d_tin, tag="tw")
                    for jj in range(2):
                        for ci in range(NCB):
                            nc.tensor.transpose(
                                pst[:, jj * CH + ci * 128:jj * CH + (ci + 1) * 128],
                                fgn[:, ci, ts(2 * jp + jj, 128)],
                                ident,
                            )
                    # evacuate the transpose PSUM; "rr" round-robins the copy
                    # across DVE/ACT/Pool so no single engine throttles the
                    # pst buffer recycling
                    fte = FTE
                    if FTE == "rr":
                        fte = ("dve", "act")[(b * (NJ // 2) + jp) % 2]
                    if fte == "pool":
                        nc.gpsimd.tensor_copy(
                            ft[:, 2 * jp:2 * jp + 2, :],
                            pst[:].rearrange("p (j c) -> p j c", j=2),
                        )
                    elif fte == "dve":
                        nc.vector.tensor_copy(
                            ft[:, 2 * jp:2 * jp + 2, :],
                            pst[:].rearrange("p (j c) -> p j c", j=2),
                        )
                    else:
                        nc.scalar.activation(
                            ft[:, 2 * jp:2 * jp + 2, :],
                            pst[:].rearrange("p (j c) -> p j c", j=2),
                            AF.Copy,
                        )
                    for jj in range(2):
                        j = 2 * jp + jj
                        scr = pool.tile([128, CH], d_ft, tag="scr")
                        if SQS == "sbuf" or (SQS == "mix" and b > 0):
                            # read the evacuated ft: pst is freed by the DVE
                            # evac alone, so PE transposes recycle pst buffers
                            # without waiting on the ACT queue
                            src = ft[:, j, :]
                        else:
                            src = pst[:, jj * CH:(jj + 1) * CH]
                        if SQ_ENG == "pool":
                            nc.gpsimd.scalar_tensor_tensor(
                                scr[:], src, 1.0, src,
                                op0=ALU.mult, op1=ALU.mult,
                                accum_out=nsq[:, j:j + 1],
                            )
                        else:
                            # read the transpose PSUM directly: the norm chain
                            # does not wait on the ft evacuation
                            nc.scalar.activation(
                                scr[:], src, AF.Square,
                                accum_out=nsq[:, j:j + 1]
                            )

                # ---- s = m * rsqrt(nsq), om = 1-m ----
                # fp8 modes: fold a x32 scale into K's quantization so K/Kr
                # stay out of e4m3's denormal range; att is descaled via om.
                nc.vector.reciprocal(rin[:], nsq[:])
                if MM1 == "f16":
                    nc.scalar.activation(rsq[:], rin[:], AF.Sqrt)
                else:
                    nc.scalar.activation(rsq[:], rin[:], AF.Sqrt, scale=1024.0)
                if NEWTON:
                    t0 = pool.tile([128, NJ], f32, tag="nt0")
                    nc.vector.tensor_mul(t0[:], rsq[:], rsq[:])
                    nc.vector.tensor_mul(t0[:], t0[:], nsq[:])
                    nc.vector.tensor_scalar(
                        t0[:], t0[:], -0.5, 1.5, ALU.mult, ALU.add
                    )
                    nc.vector.tensor_mul(rsq[:], rsq[:], t0[:])
                nc.vector.tensor_mul(s_til[:], rsq[:], m_til)
                state[("pre", b)] = (fgn, m_til, ft, s_til)

            def kq_stage(b):
                # ---- K^T = FT * s ---- (separate stage so late samples'
                # K-quant DVE ops don't queue ahead of earlier blends)
                fgn, m_til, ft, s_til = state.pop(("pre", b))
                if MM1 == "f16":
                    kt = pool.tile([128, NJ, CH], d_mm1, tag="kt")
                else:
                    # slot 0 = K8 (e4m3 of K^T), slot 1 = Kr8 (e4m3 residual)
                    kt = pool.tile([128, NJ, 2, CH], f8, tag="kt")
                if MM1 == "f16":
                    for j in range(NJ):
                        nc.vector.tensor_scalar_mul(
                            kt[:, j, :], ft[:, j, :], s_til[:, j:j + 1]
                        )
                elif KRP == "mixsub" and b > 0:
                    # Pool cannot run TensorScalarPtr, so the Pool offload
                    # goes through an f16 intermediate: DVE computes
                    # kt16 = FT*s, Pool does the e4m3 quantize (copy) and the
                    # residual subtract — both legal SBUF->SBUF TensorOps.
                    # Sample 0 keeps the all-DVE fast path (below) since its
                    # kt gates the first mm1.
                    kt16 = pool.tile([128, NJ, CH], f16, tag="kt16")
                    for j in range(NJ):
                        nc.vector.tensor_scalar_mul(
                            kt16[:, j, :], ft[:, j, :], s_til[:, j:j + 1]
                        )
                        nc.gpsimd.tensor_copy(kt[:, j, 0, :], kt16[:, j, :])
                        nc.gpsimd.tensor_sub(
                            kt[:, j, 1, :], kt16[:, j, :], kt[:, j, 0, :]
                        )
                else:
                    for j in range(NJ):
                        # K8 = e4m3(FT*s); Kr8 = e4m3(FT*s - K8), both on DVE
                        nc.vector.tensor_scalar_mul(
                            kt[:, j, 0, :], ft[:, j, :], s_til[:, j:j + 1]
                        )
                        nc.vector.scalar_tensor_tensor(
                            kt[:, j, 1, :], ft[:, j, :], s_til[:, j:j + 1],
                            kt[:, j, 0, :], op0=ALU.mult, op1=ALU.subtract,
                        )
                # blend coefficient: host prescales A by (1-m)*32 and K is
                # x32-quantized, so psum = 1024*(1-m)*att and the blend is a
                # single stt: fin' = ft*(1024 m) + psum (= 1024*fin; the
                # 1/1024 descale rides the ACT evac in fin_stage).
                m1k = pool.tile([128, NJ], f32, tag="m1k")
                if MM1 == "f16":
                    nc.vector.tensor_scalar(
                        m1k[:], m_til, -1.0, 1.0, ALU.mult, ALU.add
                    )
                    ftm = pool.tile([128, NJ, CH], d_ft, tag="ftm")
                    for j in range(NJ):
                        nc.vector.tensor_scalar_mul(
                            ftm[:, j, :], ft[:, j, :], m_til[:, j:j + 1]
                        )
                else:
                    nc.vector.tensor_scalar_mul(m1k[:], m_til, 1024.0)
                    ftm = None

                state[b] = (fgn, m_til, ft, kt, m1k, ftm)

            def w1_stage(b):
                # ---- W1@fg half of mm2: only needs fg, so it can run while
                # the PE is otherwise waiting on the A stream; fin_stage then
                # only does the W2@final half and merges via one stt.
                fgn = fgv(b)
                w1fg = pool.tile([128, 2, NCB, 512], d_ft, tag="w1fg")
                for nb in range(2):
                    for ob in range(NCB):
                        psw = psmm2.tile([128, 512], f32, tag="ps2")
                        for ic in range(NCB):
                            nc.tensor.matmul(
                                psw[:],
                                wtt[:, ic, ts(ob, 128)],
                                fgn[:, ic, ts(nb, 512)],
                                start=(ic == 0),
                                stop=(ic == NCB - 1),
                            )
                        wsl = w1fg[:, nb, ob, :]
                        if W1E == "dve":
                            nc.vector.tensor_copy(wsl, psw[:])
                        else:
                            nc.scalar.activation(wsl, psw[:], AF.Copy)
                state[("w1", b)] = w1fg

            def mm1_stage(b):
                fgn, m_til, ft, kt, m1k, ftm = state.pop(b)
                atile = state.pop(("A", b))
                # ---- mm1: att^T per pixel block; blend from PSUM ----
                # fin_t = att^T*(1-m) + ft*m (coefficients folded, see above)
                fin_t = pool.tile([128, NJ, CH], d_ft, tag="fint")
                for j in range(NJ):
                    jh, jl = j // 4, j % 4
                    psa = psatt.tile([128, CH], f32, tag="psa")
                    if MM1 == "f16":
                        for kc in range(NJ):
                            nc.tensor.matmul(
                                psa[:],
                                atile[:, jh, kc, ts(jl, 128)],
                                kt[:, kc, :],
                                start=(kc == 0),
                                stop=(kc == NJ - 1),
                            )
                    elif MM1 == "dr1":
                        # pass 1: A8 x K8 over kc pairs; pass 2: A8 x Kr8
                        for sl in range(2):
                            for kp in range(NJ // 2):
                                kc = 2 * kp
                                nc.tensor.matmul(
                                    psa[:],
                                    atile[:, jh, kc:kc + 2, ts(jl, 128)],
                                    kt[:, kc:kc + 2, sl, :],
                                    start=(sl == 0 and kp == 0),
                                    stop=(sl == 1 and kp == NJ // 2 - 1),
                                    perf_mode=DR,
                                )
                    else:
                        # pass 1: A8 x K8 over kc pairs (which-dim slot 1)
                        for kp in range(NJ // 2):
                            kc = 2 * kp
                            nc.tensor.matmul(
                                psa[:],
                                atile[:, jh, kc:kc + 2, 1, ts(jl, 128)],
                                kt[:, kc:kc + 2, 0, :],
                                start=(kp == 0),
                                stop=False,
                                perf_mode=DR,
                            )
                        # pass 2 cross: (Ar8_kc, A8_kc) x (K8_kc, Kr8_kc)
                        for kc in range(NJ):
                            nc.tensor.matmul(
                                psa[:],
                                atile[:, jh, kc, :, ts(jl, 128)],
                                kt[:, kc, :, :],
                                start=False,
                                stop=(kc == NJ - 1),
                                perf_mode=DR,
                            )
                    if MM1 == "f16":
                        nc.vector.affine_then_add(
                            fin_t[:, j, :], psa[:], ftm[:, j, :],
                            scale=m1k[:, j:j + 1], bias=0.0,
                        )
                    else:
                        nc.vector.scalar_tensor_tensor(
                            fin_t[:, j, :], ft[:, j, :], m1k[:, j:j + 1],
                            psa[:], op0=ALU.mult, op1=ALU.add,
                        )
                state[("mid", b)] = (fgn, fin_t)

            def fin_stage(b):
                fgn, fin_t = state.pop(("mid", b))
                w1fg = state.pop(("w1", b), None)

                # ---- per pixel-half: transpose final back + mm2 + out ----
                # mm2's nb-th column half only needs T-out group jg==nb, so
                # process halves end-to-end: the first half's evac/DMA then
                # overlaps the second half's PE work (shrinks the kernel tail).
                finaln = pool.tile([128, NCB, HW], d_mm2, tag="finaln")
                outsb = outsb_all[:, b]
                cats = [fgn[:, 0, :], fgn[:, 1, :], finaln[:, 0, :], finaln[:, 1, :]]
                for jg in range(2):
                    for ci in range(NCB):
                        pso = pstr.tile([128, 512], d_ft, tag="tw")
                        for jj in range(4):
                            j = jg * 4 + jj
                            nc.tensor.transpose(
                                pso[:, jj * 128:(jj + 1) * 128],
                                fin_t[:, j, ts(ci, 128)],
                                ident if DT16 else ident.bitcast(f32),
                            )
                        fsl = finaln[:, ci, jg * 512:(jg + 1) * 512]
                        fscale = 1.0 if MM1 == "f16" else 1.0 / 1024
                        if FEV == "dve":
                            nc.vector.tensor_scalar_mul(fsl, pso[:], fscale)
                        else:
                            nc.scalar.activation(
                                fsl, pso[:], AF.Copy, scale=fscale,
                            )
                    nb = jg
                    for ob in range(NCB):
                        ps2 = psmm2.tile([128, 512], f32, tag="ps2")
                        ics = range(NCB, NIC) if w1fg is not None else range(NIC)
                        for i, ic in enumerate(ics):
                            nc.tensor.matmul(
                                ps2[:],
                                wtt[:, ic, ts(ob, 128)],
                                cats[ic][:, ts(nb, 512)],
                                start=(i == 0),
                                stop=(ic == NIC - 1),
                            )
                        osl = outsb[:, nb, ob, :]
                        if w1fg is not None:
                            # out = (W2@final + bias) + W1@fg in one stt
                            nc.vector.scalar_tensor_tensor(
                                osl, ps2[:], b2t[:, ob:ob + 1],
                                w1fg[:, nb, ob, :],
                                op0=ALU.add, op1=ALU.add,
                            )
                        elif ob == 0 and OB0 == "dve":
                            nc.vector.tensor_scalar_add(
                                osl, ps2[:], b2t[:, ob:ob + 1]
                            )
                        else:
                            nc.scalar.activation(
                                osl, ps2[:], AF.Identity,
                                bias=b2t[:, ob:ob + 1],
                            )
                if b == SPC - 2:
                    nc.sync.dma_start(out_d[:, :SPC - 1], outsb_all[:, :SPC - 1])
                elif b == SPC - 1:
                    nc.sync.dma_start(out_d[:, SPC - 1:], outsb_all[:, SPC - 1:])

            # constants + all-sample fg land first (sample 0's fg split off so
            # its prep chain starts immediately); the A stream follows.
            nc.sync.dma_start(cmt[:], cm_d[:])
            if os.environ.get("TRN_DMAORD", "fgfirst") == "a0early":
                prep_A(0)
                nc.sync.dma_start(fgsb[:], fg_d[:])
                prep_A(1)
                prep_A(2)
                prep_A(3)
            else:
                nc.sync.dma_start(fgsb[:], fg_d[:])
                prep_A(0)
                prep_A(1)
                prep_A(2)
                prep_A(3)
            if True:
                # token schedule: Pb=prep, Kb=K-quant, Wb=W1@fg, Mb=mm1,
                # Fb=fin

                orders = {
                    # best of a ~1000-iter TimelineSim hill-climb over token
                    # interleavings (see ord_search.py)
                    "searched": "P0 P1 K0 W3 K1 P2 W2 P3 M0 K2 F0 M1 K3 F1 "
                                "M2 F2 M3 F3",
                    "hyb2": "P0 P1 P2 P3 K0 K1 M0 F0 K2 M1 F1 K3 M2 M3 F2 F3",
                    "hyb3": "P0 P1 K0 P2 P3 K1 M0 K2 M1 F0 K3 M2 F1 M3 F2 F3",
                    "hyb1": "P0 P1 P2 P3 K0 K1 K2 K3 M0 W1 M1 F0 M2 F1 M3 F2 F3",
                    "nw1": "P0 P1 P2 P3 K0 K1 K2 K3 M0 M1 F0 M2 F1 M3 F2 F3",
                }
                ordstr = os.environ.get("TRN_ORD") or orders.get(
                    SCHED, orders["hyb2"]
                )
                fns = {"P": prep, "K": kq_stage, "W": w1_stage,
                       "M": mm1_stage, "F": fin_stage}
                for tok in ordstr.split():
                    fns[tok[0]](int(tok[1:]))
    nc.compile()
    return nc


def _prep_inputs(foreground, mask, attention_scores, comb_w, comb_b):
    import ml_dtypes

    f32 = np.float32
    f16 = np.float16
    fdat = f16 if DT16 else f32
    # fg: [BS, 128(p), NCB, HW] partition-major, its own tensor
    fg = np.asarray(foreground, dtype=fdat).reshape(BS, NCB, 128, HW)
    fg = np.ascontiguousarray(fg.transpose(0, 2, 1, 3))   # [BS, 128, NCB, HW]
    atf = np.asarray(attention_scores, dtype=f32).reshape(BS, HW, HW)
    m_pre = np.asarray(mask, dtype=f32).reshape(BS, 1, HW)
    if MM1 != "f16":
        # fold the blend coefficient into A: columns scaled by (1-m_p)*32
        atf = atf * ((1.0 - m_pre) * 32.0)
    if MM1 == "dr2":
        a8 = atf.astype(ml_dtypes.float8_e4m3)
        ar8 = (atf - a8.astype(f32)).astype(ml_dtypes.float8_e4m3)
        at = np.stack([ar8, a8], axis=2)        # [BS, HW(krow), 2, HW]
        # -> [BS, p, jhalf, kchunk(8), which, 512pix]: one run per partition,
        # pixel-half-outer so the first half of a sample's DMA already covers
        # output blocks j=0..3 for every k chunk
        at = at.reshape(BS, 8, 128, 2, 2, 512).transpose(0, 2, 4, 1, 3, 5)
        abody = np.ascontiguousarray(at).reshape(BS, 128, 8 * 2 * HW)
    elif MM1 == "dr1":
        at = atf.astype(ml_dtypes.float8_e4m3)
        at = at.reshape(BS, 8, 128, 2, 512).transpose(0, 2, 3, 1, 4)
        abody = np.ascontiguousarray(at).reshape(BS, 128, 8 * HW)
    else:
        at = np.asarray(atf, dtype=fdat)
        at = at.reshape(BS, 8, 128, 2, 512).transpose(0, 2, 3, 1, 4)
        abody = np.ascontiguousarray(at).reshape(
            BS, 128, 8 * HW * 2).view(np.uint8)
    m = np.asarray(mask, dtype=f32).reshape(BS, HW)
    mt = m.reshape(BS, NJ, 128).transpose(0, 2, 1)   # [BS, 128, NJ]
    wt = np.asarray(comb_w, dtype=fdat).T          # [512, 256]
    wtr = wt.reshape(NIC, 128, CH).transpose(1, 0, 2).reshape(128, NIC * CH)
    cst = np.ascontiguousarray(wtr.astype(fdat))
    ident = np.eye(128, dtype=fdat)
    b2 = np.ascontiguousarray(
        np.asarray(comb_b, dtype=f32).reshape(NCB, 128).T
    )
    in_maps = []
    for c in range(NCORES):
        sl = slice(c * SPC, (c + 1) * SPC)
        # merged const block: ident | comb_w^T | b2(bitcast f16) | mask
        mcore = np.ascontiguousarray(
            mt[sl].transpose(1, 0, 2).reshape(128, SPC * NJ)
        )
        fgc = fg[sl]                      # [SPC, 128, NCB, HW]
        # sample 0's fg rides behind the constants in the cm tensor
        fg0cols = fgc[0].reshape(128, NCB * HW).view(f16)
        cm = np.concatenate(
            [ident, cst, b2.view(f16), mcore.view(f16), fg0cols], axis=1
        )
        in_maps.append({
            "attn": np.ascontiguousarray(abody[sl]),
            "fgd": np.ascontiguousarray(fgc[1:].transpose(1, 0, 2, 3)),
            "cm": np.ascontiguousarray(cm),
        })
    return in_maps


def _unshard_out(arr):
    """[ncores*128, SPC, 2, NCB, 512] partition-major layout -> [N, CH, H, W]."""
    arr = np.asarray(arr, dtype=np.float32)
    nc_ = arr.shape[0] // 128
    return (arr.reshape(nc_, 128, SPC, 2, NCB, 512)
            .transpose(0, 2, 4, 1, 3, 5)
            .reshape(nc_ * SPC, CH, H, W))

def run(inputs, trace=False):
    from concourse.bass_utils import run_bass_kernel_spmd

    if "nc" not in _cache:
        _cache["nc"] = _build()
    nc = _cache["nc"]
    in_maps = _prep_inputs(**inputs)
    res = run_bass_kernel_spmd(
        nc, in_maps, core_ids=list(range(NCORES)), trace=trace,
    )
    outs = [_unshard_out(r["out"]) for r in res.results]
    full = np.concatenate(outs, axis=0)
    return full, res


def kernel(**inputs) -> np.ndarray:
    out, _ = run(inputs, trace=False)
    return out


def bench(inputs, iters=20):
    """Build the sharded PJRT executable once, keep inputs device-resident,
    and time repeated executions (min over iters). Mirrors
    bass2jax.run_bass_via_pjrt's multi-core path without per-call retracing."""
    import time

    import jax
    import concourse.mybir as mybir
    from concourse.bass2jax import (
        _bass_exec_p,
        install_neuronx_cc_hook,
        partition_id_tensor,
        Mesh,
        PartitionSpec,
        shard_map,
    )

    install_neuronx_cc_hook()
    if "nc" not in _cache:
        _cache["nc"] = _build()
    nc = _cache["nc"]
    in_maps = _prep_inputs(**inputs)

    partition_name = (
        nc.partition_id_tensor.name if nc.partition_id_tensor else None
    )
    in_names, out_names, out_avals, zero_outs = [], [], [], []
    for alloc in nc.m.functions[0].allocations:
        if not isinstance(alloc, mybir.MemoryLocationSet):
            continue
        name = alloc.memorylocations[0].name
        if alloc.kind == "ExternalInput":
            if name != partition_name:
                in_names.append(name)
        elif alloc.kind == "ExternalOutput":
            out_names.append(name)
            shape = tuple(alloc.tensor_shape)
            dtype = mybir.dt.np(alloc.dtype)
            out_avals.append(jax.core.ShapedArray(shape, dtype))
            zero_outs.append(np.zeros(shape, dtype))
    n_params = len(in_names)
    all_in_names = in_names + out_names
    if partition_name is not None:
        all_in_names = all_in_names + [partition_name]

    def _body(*args):
        operands = list(args)
        if partition_name is not None:
            operands.append(partition_id_tensor())
        outs = _bass_exec_p.bind(
            *operands,
            out_avals=tuple(out_avals),
            in_names=tuple(all_in_names),
            out_names=tuple(out_names),
            lowering_input_output_aliases=(),
            sim_require_finite=True,
            sim_require_nnan=True,
            nc=nc,
        )
        return tuple(outs)

    devices = jax.devices()[:NCORES]
    mesh = Mesh(np.asarray(devices), ("core",))
    in_specs = (PartitionSpec("core"),) * (n_params + len(out_names))
    out_specs = (PartitionSpec("core"),) * len(out_names)
    sharded = jax.jit(
        shard_map(_body, mesh=mesh, in_specs=in_specs, out_specs=out_specs,
                  check_rep=False),
        keep_unused=True,
    )
    concat_in = [
        np.concatenate([in_maps[c][nm] for c in range(NCORES)], axis=0)
        for nm in in_names
    ]
    concat_zeros = [
        np.zeros((NCORES * z.shape[0], *z.shape[1:]), z.dtype) for z in zero_outs
    ]
    sharding = jax.sharding.NamedSharding(mesh, PartitionSpec("core"))
    dev_in = [jax.device_put(x, sharding) for x in concat_in]
    dev_zero = [jax.device_put(x, sharding) for x in concat_zeros]

    # warmup (compiles)
    out = sharded(*dev_in, *dev_zero)
    jax.block_until_ready(out)
    times = []
    for _ in range(iters):
        t0 = time.perf_counter()
        out = sharded(*dev_in, *dev_zero)
        jax.block_until_ready(out)
        times.append(time.perf_counter() - t0)
    full = _unshard_out(out[0])
    return full, times


def bench_chain(inputs, n_chain=64, iters=8):
    """Time N chained NEFF executions inside one dispatch; the slope
    (T_chain - T_single)/(n_chain-1) removes the ~40-80ms axon RPC overhead."""
    import time

    import jax
    import concourse.mybir as mybir
    from concourse.bass2jax import (
        _bass_exec_p,
        install_neuronx_cc_hook,
        partition_id_tensor,
        Mesh,
        PartitionSpec,
        shard_map,
    )

    install_neuronx_cc_hook()
    if "nc" not in _cache:
        _cache["nc"] = _build()
    nc = _cache["nc"]
    in_maps = _prep_inputs(**inputs)

    partition_name = (
        nc.partition_id_tensor.name if nc.partition_id_tensor else None
    )
    in_names, out_names, out_avals, zero_outs = [], [], [], []
    for alloc in nc.m.functions[0].allocations:
        if not isinstance(alloc, mybir.MemoryLocationSet):
            continue
        name = alloc.memorylocations[0].name
        if alloc.kind == "ExternalInput":
            if name != partition_name:
                in_names.append(name)
        elif alloc.kind == "ExternalOutput":
            out_names.append(name)
            shape = tuple(alloc.tensor_shape)
            dtype = mybir.dt.np(alloc.dtype)
            out_avals.append(jax.core.ShapedArray(shape, dtype))
            zero_outs.append(np.zeros(shape, dtype))
    n_params = len(in_names)
    all_in_names = in_names + out_names
    if partition_name is not None:
        all_in_names = all_in_names + [partition_name]

    def _body(*args):
        operands = list(args)
        if partition_name is not None:
            operands.append(partition_id_tensor())
        return tuple(_bass_exec_p.bind(
            *operands,
            out_avals=tuple(out_avals),
            in_names=tuple(all_in_names),
            out_names=tuple(out_names),
            lowering_input_output_aliases=(),
            sim_require_finite=True,
            sim_require_nnan=True,
            nc=nc,
        ))

    devices = jax.devices()[:NCORES]
    mesh = Mesh(np.asarray(devices), ("core",))
    in_specs = (PartitionSpec("core"),) * (n_params + len(out_names))
    out_specs = (PartitionSpec("core"),) * len(out_names)
    sharded = jax.jit(
        shard_map(_body, mesh=mesh, in_specs=in_specs,
                  out_specs=out_specs, check_rep=False),
        keep_unused=True,
    )

    concat_in = [
        np.concatenate([in_maps[c][nm] for c in range(NCORES)], axis=0)
        for nm in in_names
    ]
    concat_zeros = [
        np.zeros((NCORES * z.shape[0], *z.shape[1:]), z.dtype) for z in zero_outs
    ]
    sharding = jax.sharding.NamedSharding(mesh, PartitionSpec("core"))
    dev_in = [jax.device_put(x, sharding) for x in concat_in]
    dev_zero = [jax.device_put(x, sharding) for x in concat_zeros]

    def run_n(n):
        # async-dispatch n executions, chained through the donated output
        # buffers so they serialize on-device; block once at the end
        outs = dev_zero
        for _ in range(n):
            outs = list(sharded(*dev_in, *outs))
        jax.block_until_ready(outs)
        return outs

    out = run_n(1)  # warmup / compile

    def timed_once(n):
        t0 = time.perf_counter()
        run_n(n)
        return time.perf_counter() - t0

    # The axon-tunneled device sees interference windows from other tenants
    # plus per-call RPC jitter; interference only ever adds time, so
    # interleave repeated timings of both chain lengths across spaced-out
    # rounds and take per-endpoint minima. Deep chains (~128) give a 50ms+
    # slope signal that dwarfs the +-5ms RPC jitter.
    n1 = max(2, n_chain // 16)
    t1, tn = 1e18, 1e18
    rounds = max(8, iters // 2)
    for r in range(rounds):
        if r:
            time.sleep(0.7)
        for _ in range(2):
            t1 = min(t1, timed_once(n1))
        # the long chain is the volatile endpoint (its floor swings with the
        # dispatch-pipeline phase) — weight the sampling toward it
        for _ in range(4):
            tn = min(tn, timed_once(n_chain))
    out = run_n(1)
    per_exec = (tn - t1) / (n_chain - n1)
    full = _unshard_out(out[0])
    return full, per_exec, (t1, tn)


# revision 54
# speedup vs baseline: 1.4310x; 1.4310x over previous
"""Trainium2 Bass kernel for nn_AttentionMergeMask (8 NeuronCores, SPMD).

Reference computation (per sample b):
    K[c,k]   = (fg[c,k]+EPS) / ||fg[:,k]+EPS|| * m[k]        (k = pixel idx, 1024)
    att[c,p] = sum_k K[c,k] * A[k,p]                          (A = attention_scores[b])
    final    = att*(1-m) + fg*m
    out      = comb_w @ [fg; final] + comb_b

Sharding: pure data parallel — batch 32 split 4 samples per core across the
8 cores; small combiner weights replicated.

Per-core kernel (Tile framework):
  - work in a transposed [pixel, channel] layout so every norm/mask step is a
    per-partition scalar op:
      PE-transpose fg -> FT[pix,ch] (PSUM); ACT Square+accum_out reads the
      PSUM directly for normsq[pix]; s = 32*m*sqrt(1/normsq) (the x32 keeps
      the fp8 K quantization out of e4m3's denormal range)
  - mm1 in fp8 e4m3 with the DoubleRow perf mode (2 contraction rows per
    PE pass, 0.5 cycles/row): the big attention matmul runs 2x faster than
    fp16. Accuracy is restored by residual ("split") quantization:
      A is host-prescaled by the blend coefficient (1-m_p)*32 and shipped as
      A8 + Ar8 (e4m3 value + e4m3 residual, same bytes as fp16); K^T is
      quantized on-device as K8 + Kr8 (DVE + Pool writes).
      mm1 accumulates A8@K8 (4 DoubleRow instrs) plus the cross terms
      Ar8@K8 + A8@Kr8 (8 DoubleRow instrs, pairing the two matrices per
      instruction); only the doubly-small Ar8@Kr8 term is dropped.
  - blend is ONE DVE scalar_tensor_tensor per block: since A carries
    (1-m)*32 and K carries x32, psum = 1024*(1-m)*att, so
    fin' = ft*(1024*m) + psum = 1024*final; the 1/1024 descale rides the
    ACT Copy evacuation of the back-transpose for free.
  - PE-transpose fin' back to natural; mm2 (fp16) out = W^T @ [fg; final]
    with host-pretransposed weights; comb_b added during PSUM evacuation.

DMA/overlap structure (v2): fg for ALL samples lands in the first ~7us
(sample 0's fg rides behind the constants in the cm tensor so the whole
early block is one DMA); the attention stream (A8+Ar8, 16KB/partition/
sample, pixel-half-outer layout) then streams behind it. All four samples'
prep chains (transposes, norms, K-quant) complete DURING the ~29us A stream
instead of trailing it. Stage emission is token-driven (P/K/W/M/F per
sample) and the default order is the best of a ~2000-eval TimelineSim
hill-climb (ord_search*.py) over interleavings + engine-assignment knobs;
K-quant is a separate token so late samples' DVE ops don't queue ahead of
earlier samples' blends (engine queues execute in emission order). Engine
notes: Pool/GPSIMD cannot read PSUM and cannot run TensorScalarPtr on TRN2
hardware (BIR verifier), so every PSUM evacuation sits on DVE/ACT and the
Pool offload of the Kr8 residual goes through an f16 intermediate
(DVE kt16 -> Pool e4m3-copy + subtract). The W1@fg half of mm2 (fg-only)
precomputes for samples 2-3 during A-stream gaps, and fin merges it with
W2@final + bias in one DVE stt. Cost-model (TimelineSim) per-core exec
dropped 72.1us -> 61.3us; all engines sit at 52-70% busy with PE and ACT
co-limiting (remaining gap is in-order engine-queue coupling).

I/O layouts are host-pre-permuted partition-major so every DMA transfer is
one large contiguous run per partition (128 descriptors): per-exec
descriptor count ~1k (the axon PJRT path charges ~50ns/descriptor).

Numerics: EPS dropped (relative impact ~1e-7 for unit-scale randn inputs).
Measured on-hw output error vs the fp32 reference: ~7e-4 norm-relative
(TRN_MM1=dr2 default), allclose(2e-2) True. TRN_MM1=dr1 additionally drops
Ar8 (halves attn DMA bytes, mm1 in 8 instrs) at 1.33e-2 error but fails the
elementwise allclose(2e-2); TRN_MM1=f16 is the fp16 path (~4.2e-4, no fp8).
"""

import os
import numpy as np

NCORES = 8
BS, CH, H, W = 32, 256, 32, 32
HW = H * W                     # 1024
SPC = BS // NCORES             # samples per core = 4
NJ = HW // 128                 # 8 pixel chunks
NCB = CH // 128                # 2 channel blocks
NIC = (2 * CH) // 128          # 4 cat chunks

MM_F32R = os.environ.get("TRN_MM_F32R", "1") == "1"      # mm1 (A @ K^T)
MM2_F32R = os.environ.get("TRN_MM2_F32R", "1") == "1"    # mm2 (W @ cat)
TIN_F32R = os.environ.get("TRN_TIN_F32R", "0") == "1"    # fg transposes
NEWTON = os.environ.get("TRN_NEWTON", "0") == "1"
DT16 = os.environ.get("TRN_DT16", "1") == "1"    # fp16 data path (A, fg, W, K^T)
# mm1 mode: "dr2" = fp8 DoubleRow with full residual correction (A=A8+Ar8,
# K=K8+Kr8, dropping only the tiny Ar8@Kr8 cross term; fp16-level accuracy),
# "dr1" = fp8 DoubleRow, A quantized to a single e4m3 level (K keeps its
# residual); halves A DMA traffic at ~1.3e-2 rel err. "f16" = original path.
MM1 = os.environ.get("TRN_MM1", "dr2")
SQ_ENG = os.environ.get("TRN_SQ", "act")         # norm-square engine
SQS = os.environ.get("TRN_SQS", "mix")           # norm-square source: psum|sbuf|mix
FTE = os.environ.get("TRN_FTE", "dve")           # ft-evac engine: pool|dve|act
KRP = os.environ.get("TRN_KRP", "mixsub")        # Kr8 residual: mixsub|dve
OB0 = os.environ.get("TRN_OB0", "act")           # mm2 ob0 evac: dve|act
SCHED = os.environ.get("TRN_SCHED", "searched")  # stage order
PSB = os.environ.get("TRN_PSB", "332")           # psum bufs: pstr,psa,ps2
PEW = int(os.environ.get("TRN_PEW", "8"))        # PE p-state warm matmuls
FEV = os.environ.get("TRN_FEV", "act")           # finaln evac: act|pool
# mm2 W1@fg half: "prep" precomputes it during the prep phase (hidden under
# the A DMA stream; fin only does W2@final and merges via one stt per block),
# "fin" computes the full contraction in fin_stage (legacy).
W1P = os.environ.get("TRN_W1P", "prep")
# NOTE: GPSIMD/Pool cannot read PSUM on TRN2 (BIR verifier rejects it), so
# every PSUM evacuation must be on DVE or ACT; Pool only gets SBUF->SBUF ops.
W1E = os.environ.get("TRN_W1E", "act")           # W1@fg evac engine: act|dve

_cache = {}


def _build():
    import concourse.bass as bass
    import concourse.tile as tile
    import concourse.mybir as mybir
    from concourse import bacc
    from concourse.bass import ts

    f32 = mybir.dt.float32
    f32r = mybir.dt.float32r
    f16 = mybir.dt.float16
    f8 = mybir.dt.float8e4
    DR = mybir.MatmulPerfMode.DoubleRow
    if DT16:
        d_mm1 = d_mm2 = d_tin = f16
        d_ft = f16      # FT / scratch / blend tiles
    else:
        d_mm1 = f32r if MM_F32R else f32    # ah, kt
        d_mm2 = f32r if MM2_F32R else f32   # wtt, fgn(mm2 rhs), finaln
        d_tin = f32r if TIN_F32R else f32   # fgn/ident/pst transpose path
        d_ft = f32
    AF = mybir.ActivationFunctionType
    ALU = mybir.AluOpType

    nc = bacc.Bacc(
        "TRN2",
        target_bir_lowering=False,
        debug=False,
        enable_asserts=False,
    )
    # All inputs are host-pre-permuted to partition-major layouts so every
    # DMA moves one large contiguous run per partition (128 descriptors per
    # transfer instead of 1024): the axon path costs ~50ns per descriptor
    # per execution.
    if MM1 == "dr2":
        # innermost per k-row: [Ar8 | A8] so cross-pass lhsT pairs are
        # contiguous; layout [b, p, kchunk(8)*which*pix]
        ABODY = 8 * 2 * HW
        at_d = nc.dram_tensor("attn", [SPC, 128, ABODY], f8,
                              kind="ExternalInput")
    elif MM1 == "dr1":
        ABODY = 8 * HW
        at_d = nc.dram_tensor("attn", [SPC, 128, ABODY], f8,
                              kind="ExternalInput")
    else:
        ABODY = 8 * HW
        at_d = nc.dram_tensor("attn", [SPC, 128, ABODY * 2], mybir.dt.uint8,
                              kind="ExternalInput")
    # fg for all samples, partition-major: [p, b, cblock, pix]
    fg_d = nc.dram_tensor("fgd", [128, SPC - 1, NCB, HW], d_tin,
                          kind="ExternalInput")
    # merged constants: [ identity(128) | comb_w^T blocks (NIC*CH) |
    #                    comb_b bit-packed as f16 pairs (2*NCB) | mask (SPC*NJ
    #                    packed as f32 bit-pairs -> 2*SPC*NJ f16 cols) ]
    NCONST = 128 + NIC * CH + 2 * NCB + 2 * SPC * NJ
    # sample 0's fg rides behind the constants so the whole early block is
    # ONE DMA (128 descriptors) and the A stream starts a transfer earlier
    NCMFG = NCONST + NCB * HW * (1 if DT16 else 2)
    cm_d = nc.dram_tensor("cm", [128, NCMFG], f16, kind="ExternalInput")
    # out layout [p, b, pixhalf, ochunk, 512]: batched end-of-kernel DMAs with
    # a large contiguous run per partition
    out_d = nc.dram_tensor("out", [128, SPC, 2, NCB, 512],
                           f16 if DT16 else f32, kind="ExternalOutput")

    with tile.TileContext(nc) as tc:
        with (
            tc.tile_pool(name="const", bufs=1) as cpool,
            tc.tile_pool(name="sb", bufs=4 if DT16 else 2) as pool,
            tc.tile_pool(name="abuf", bufs=4) as apool,
            tc.tile_pool(name="pst", bufs=int(PSB[0]),
                         space=bass.MemorySpace.PSUM) as pstr,
            tc.tile_pool(name="psa", bufs=int(PSB[1]),
                         space=bass.MemorySpace.PSUM) as psatt,
            tc.tile_pool(name="ps2", bufs=int(PSB[2]),
                         space=bass.MemorySpace.PSUM) as psmm2,
        ):
            cmt = cpool.tile([128, NCMFG], f16)
            ident = cmt[:, 0:128]
            cst = cmt[:, 128:128 + NIC * CH]
            wtt = cst.rearrange("p (ic o) -> p ic o", ic=NIC)
            _o = 128 + NIC * CH
            b2t = cmt[:, _o:_o + 2 * NCB].bitcast(f32)
            _o += 2 * NCB
            m_all = cmt[:, _o:_o + 2 * SPC * NJ].bitcast(f32).rearrange(
                "p (b j) -> p b j", b=SPC
            )
            fg0sb = cmt[:, NCONST:].bitcast(d_tin).rearrange(
                "p (c f) -> p c f", c=NCB
            )
            fgsb = cpool.tile([128, SPC - 1, NCB, HW], d_tin)

            def fgv(b):
                return fg0sb if b == 0 else fgsb[:, b - 1]
            outsb_all = cpool.tile([128, SPC, 2, NCB, 512], d_ft)
            # Pre-warm the ACT spline-table sets (Square, Sqrt) at t=0 so the
            # ~2.6us table loads stay off the first sample's norm chain.
            warm = cpool.tile([128, 1], f32)
            nc.gpsimd.memset(warm[:], 1.0)
            nc.scalar.activation(warm[:], warm[:], AF.Square)
            nc.scalar.activation(warm[:], warm[:], AF.Sqrt)
            if PEW:
                # PE p-state warm-up: the tensor engine clocks up only after
                # sustained execution (0.65 -> 1.2 -> 2.4 GHz); burn dummy
                # matmuls during the initial DMA wait so sample 0's
                # transposes and first mm1 groups run at full clock.
                pwd = cpool.tile([128, 256], d_tin)
                nc.gpsimd.memset(pwd[:], 1.0)
                pwps = psatt.tile([128, CH], f32, tag="psa", name="pwps")
                for _ in range(PEW):
                    nc.tensor.matmul(
                        pwps[:], pwd[:, 0:128], pwd[:],
                        start=True, stop=True,
                    )
            state = {}

            def prep_A(b):
                # one DMA per sample covering A (128 descriptors); sample 0
                # splits into pixel halves so mm1(0) starts on the first half
                # ~3us before the full transfer lands. Layout is jhalf-outer:
                # [p, jhalf, kchunk, (which,) 512pix].
                if MM1 == "f16":
                    raw = apool.tile([128, ABODY * 2], mybir.dt.uint8,
                                     tag="A")
                    a = raw[:].bitcast(d_mm1).rearrange(
                        "p (h k f) -> p h k f", h=2, k=8
                    )
                else:
                    raw = apool.tile([128, ABODY], f8, tag="A")
                    if MM1 == "dr2":
                        a = raw[:].rearrange(
                            "p (h k two f) -> p h k two f", h=2, k=8, two=2
                        )
                    else:
                        a = raw[:].rearrange(
                            "p (h k f) -> p h k f", h=2, k=8
                        )
                if b == 0 and os.environ.get("TRN_ASPL", "0") == "1":
                    nbytes = raw.shape[1] // 2
                    nc.sync.dma_start(raw[:, :nbytes], at_d[b, :, :nbytes])
                    nc.sync.dma_start(raw[:, nbytes:], at_d[b, :, nbytes:])
                else:
                    nc.sync.dma_start(raw[:], at_d[b][:])
                state[("A", b)] = a

            def prep(b):
                fgn = fgv(b)
                m_til = m_all[:, b, :]

                # ---- transpose fg -> FT[pix, ch], normsq via Square+accum ----
                ft = pool.tile([128, NJ, CH], d_ft, tag="ft")
                nsq = pool.tile([128, NJ], f32, tag="nsq")
                rin = pool.tile([128, NJ], f32, tag="rin")
                rsq = pool.tile([128, NJ], f32, tag="rsq")
                s_til = pool.tile([128, NJ], f32, tag="stil")
                for jp in range(NJ // 2):
                    pst = pstr.tile([128, 2 * CH], d_tin, tag="tw")
                    for jj in range(2):
                        for ci in range(NCB):
                            nc.tensor.transpose(
                                pst[:, jj * CH + ci * 128:jj * CH + (ci + 1) * 128],
                                fgn[:, ci, ts(2 * jp + jj, 128)],
                                ident,
                            )
                    # evacuate the transpose PSUM; "rr" round-robins the copy
                    # across DVE/ACT/Pool so no single engine throttles the
                    # pst buffer recycling
                    fte = FTE
                    if FTE == "rr":
                        fte = ("dve", "act")[(b * (NJ // 2) + jp) % 2]
                    if fte == "pool":
                        nc.gpsimd.tensor_copy(
                            ft[:, 2 * jp:2 * jp + 2, :],
                            pst[:].rearrange("p (j c) -> p j c", j=2),
                        )
                    elif fte == "dve":
                        nc.vector.tensor_copy(
                            ft[:, 2 * jp:2 * jp + 2, :],
                            pst[:].rearrange("p (j c) -> p j c", j=2),
                        )
                    else:
                        nc.scalar.activation(
                            ft[:, 2 * jp:2 * jp + 2, :],
                            pst[:].rearrange("p (j c) -> p j c", j=2),
                            AF.Copy,
                        )
                    for jj in range(2):
                        j = 2 * jp + jj
                        scr = pool.tile([128, CH], d_ft, tag="scr")
                        if SQS == "sbuf" or (SQS == "mix" and b > 0):
                            # read the evacuated ft: pst is freed by the DVE
                            # evac alone, so PE transposes recycle pst buffers
                            # without waiting on the ACT queue
                            src = ft[:, j, :]
                        else:
                            src = pst[:, jj * CH:(jj + 1) * CH]
                        if SQ_ENG == "pool":
                            nc.gpsimd.scalar_tensor_tensor(
                                scr[:], src, 1.0, src,
                                op0=ALU.mult, op1=ALU.mult,
                                accum_out=nsq[:, j:j + 1],
                            )
                        else:
                            # read the transpose PSUM directly: the norm chain
                            # does not wait on the ft evacuation
                            nc.scalar.activation(
                                scr[:], src, AF.Square,
                                accum_out=nsq[:, j:j + 1]
                            )

                # ---- s = m * rsqrt(nsq), om = 1-m ----
                # fp8 modes: fold a x32 scale into K's quantization so K/Kr
                # stay out of e4m3's denormal range; att is descaled via om.
                nc.vector.reciprocal(rin[:], nsq[:])
                if MM1 == "f16":
                    nc.scalar.activation(rsq[:], rin[:], AF.Sqrt)
                else:
                    nc.scalar.activation(rsq[:], rin[:], AF.Sqrt, scale=1024.0)
                if NEWTON:
                    t0 = pool.tile([128, NJ], f32, tag="nt0")
                    nc.vector.tensor_mul(t0[:], rsq[:], rsq[:])
                    nc.vector.tensor_mul(t0[:], t0[:], nsq[:])
                    nc.vector.tensor_scalar(
                        t0[:], t0[:], -0.5, 1.5, ALU.mult, ALU.add
                    )
                    nc.vector.tensor_mul(rsq[:], rsq[:], t0[:])
                nc.vector.tensor_mul(s_til[:], rsq[:], m_til)
                state[("pre", b)] = (fgn, m_til, ft, s_til)

            def kq_stage(b):
                # ---- K^T = FT * s ---- (separate stage so late samples'
                # K-quant DVE ops don't queue ahead of earlier blends)
                fgn, m_til, ft, s_til = state.pop(("pre", b))
                if MM1 == "f16":
                    kt = pool.tile([128, NJ, CH], d_mm1, tag="kt")
                else:
                    # slot 0 = K8 (e4m3 of K^T), slot 1 = Kr8 (e4m3 residual)
                    kt = pool.tile([128, NJ, 2, CH], f8, tag="kt")
                if MM1 == "f16":
                    for j in range(NJ):
                        nc.vector.tensor_scalar_mul(
                            kt[:, j, :], ft[:, j, :], s_til[:, j:j + 1]
                        )
                elif KRP == "mixsub" and b > 0:
                    # Pool cannot run TensorScalarPtr, so the Pool offload
                    # goes through an f16 intermediate: DVE computes
                    # kt16 = FT*s, Pool does the e4m3 quantize (copy) and the
                    # residual subtract — both legal SBUF->SBUF TensorOps.
                    # Sample 0 keeps the all-DVE fast path (below) since its
                    # kt gates the first mm1.
                    kt16 = pool.tile([128, NJ, CH], f16, tag="kt16")
                    for j in range(NJ):
                        nc.vector.tensor_scalar_mul(
                            kt16[:, j, :], ft[:, j, :], s_til[:, j:j + 1]
                        )
                        nc.gpsimd.tensor_copy(kt[:, j, 0, :], kt16[:, j, :])
                        nc.gpsimd.tensor_sub(
                            kt[:, j, 1, :], kt16[:, j, :], kt[:, j, 0, :]
                        )
                else:
                    for j in range(NJ):
                        # K8 = e4m3(FT*s); Kr8 = e4m3(FT*s - K8), both on DVE
                        nc.vector.tensor_scalar_mul(
                            kt[:, j, 0, :], ft[:, j, :], s_til[:, j:j + 1]
                        )
                        nc.vector.scalar_tensor_tensor(
                            kt[:, j, 1, :], ft[:, j, :], s_til[:, j:j + 1],
                            kt[:, j, 0, :], op0=ALU.mult, op1=ALU.subtract,
                        )
                # blend coefficient: host prescales A by (1-m)*32 and K is
                # x32-quantized, so psum = 1024*(1-m)*att and the blend is a
                # single stt: fin' = ft*(1024 m) + psum (= 1024*fin; the
                # 1/1024 descale rides the ACT evac in fin_stage).
                m1k = pool.tile([128, NJ], f32, tag="m1k")
                if MM1 == "f16":
                    nc.vector.tensor_scalar(
                        m1k[:], m_til, -1.0, 1.0, ALU.mult, ALU.add
                    )
                    ftm = pool.tile([128, NJ, CH], d_ft, tag="ftm")
                    for j in range(NJ):
                        nc.vector.tensor_scalar_mul(
                            ftm[:, j, :], ft[:, j, :], m_til[:, j:j + 1]
                        )
                else:
                    nc.vector.tensor_scalar_mul(m1k[:], m_til, 1024.0)
                    ftm = None

                state[b] = (fgn, m_til, ft, kt, m1k, ftm)

            def w1_stage(b):
                # ---- W1@fg half of mm2: only needs fg, so it can run while
                # the PE is otherwise waiting on the A stream; fin_stage then
                # only does the W2@final half and merges via one stt.
                fgn = fgv(b)
                w1fg = pool.tile([128, 2, NCB, 512], d_ft, tag="w1fg")
                for nb in range(2):
                    for ob in range(NCB):
                        psw = psmm2.tile([128, 512], f32, tag="ps2")
                        for ic in range(NCB):
                            nc.tensor.matmul(
                                psw[:],
                                wtt[:, ic, ts(ob, 128)],
                                fgn[:, ic, ts(nb, 512)],
                                start=(ic == 0),
                                stop=(ic == NCB - 1),
                            )
                        wsl = w1fg[:, nb, ob, :]
                        if W1E == "dve":
                            nc.vector.tensor_copy(wsl, psw[:])
                        else:
                            nc.scalar.activation(wsl, psw[:], AF.Copy)
                state[("w1", b)] = w1fg

            def mm1_stage(b):
                fgn, m_til, ft, kt, m1k, ftm = state.pop(b)
                atile = state.pop(("A", b))
                # ---- mm1: att^T per pixel block; blend from PSUM ----
                # fin_t = att^T*(1-m) + ft*m (coefficients folded, see above)
                fin_t = pool.tile([128, NJ, CH], d_ft, tag="fint")
                for j in range(NJ):
                    jh, jl = j // 4, j % 4
                    psa = psatt.tile([128, CH], f32, tag="psa")
                    if MM1 == "f16":
                        for kc in range(NJ):
                            nc.tensor.matmul(
                                psa[:],
                                atile[:, jh, kc, ts(jl, 128)],
                                kt[:, kc, :],
                                start=(kc == 0),
                                stop=(kc == NJ - 1),
                            )
                    elif MM1 == "dr1":
                        # pass 1: A8 x K8 over kc pairs; pass 2: A8 x Kr8
                        for sl in range(2):
                            for kp in range(NJ // 2):
                                kc = 2 * kp
                                nc.tensor.matmul(
                                    psa[:],
                                    atile[:, jh, kc:kc + 2, ts(jl, 128)],
                                    kt[:, kc:kc + 2, sl, :],
                                    start=(sl == 0 and kp == 0),
                                    stop=(sl == 1 and kp == NJ // 2 - 1),
                                    perf_mode=DR,
                                )
                    else:
                        # pass 1: A8 x K8 over kc pairs (which-dim slot 1)
                        for kp in range(NJ // 2):
                            kc = 2 * kp
                            nc.tensor.matmul(
                                psa[:],
                                atile[:, jh, kc:kc + 2, 1, ts(jl, 128)],
                                kt[:, kc:kc + 2, 0, :],
                                start=(kp == 0),
                                stop=False,
                                perf_mode=DR,
                            )
                        # pass 2 cross: (Ar8_kc, A8_kc) x (K8_kc, Kr8_kc)
                        for kc in range(NJ):
                            nc.tensor.matmul(
                                psa[:],
                                atile[:, jh, kc, :, ts(jl, 128)],
                                kt[:, kc, :, :],
                                start=False,
                                stop=(kc == NJ - 1),
                                perf_mode=DR,
                            )
                    if MM1 == "f16":
                        nc.vector.affine_then_add(
                            fin_t[:, j, :], psa[:], ftm[:, j, :],
                            scale=m1k[:, j:j + 1], bias=0.0,
                        )
                    else:
                        nc.vector.scalar_tensor_tensor(
                            fin_t[:, j, :], ft[:, j, :], m1k[:, j:j + 1],
                            psa[:], op0=ALU.mult, op1=ALU.add,
                        )
                state[("mid", b)] = (fgn, fin_t)

            def fin_stage(b):
                fgn, fin_t = state.pop(("mid", b))
                w1fg = state.pop(("w1", b), None)

                # ---- per pixel-half: transpose final back + mm2 + out ----
                # mm2's nb-th column half only needs T-out group jg==nb, so
                # process halves end-to-end: the first half's evac/DMA then
                # overlaps the second half's PE work (shrinks the kernel tail).
                finaln = pool.tile([128, NCB, HW], d_mm2, tag="finaln")
                outsb = outsb_all[:, b]
                cats = [fgn[:, 0, :], fgn[:, 1, :], finaln[:, 0, :], finaln[:, 1, :]]
                for jg in range(2):
                    for ci in range(NCB):
                        pso = pstr.tile([128, 512], d_ft, tag="tw")
                        for jj in range(4):
                            j = jg * 4 + jj
                            nc.tensor.transpose(
                                pso[:, jj * 128:(jj + 1) * 128],
                                fin_t[:, j, ts(ci, 128)],
                                ident if DT16 else ident.bitcast(f32),
                            )
                        fsl = finaln[:, ci, jg * 512:(jg + 1) * 512]
                        fscale = 1.0 if MM1 == "f16" else 1.0 / 1024
                        fev = FEV
                        if FEV == "rr":
                            fev = ("act", "dve")[(2 * b + jg) % 2]
                        if fev == "dve":
                            nc.vector.tensor_scalar_mul(fsl, pso[:], fscale)
                        else:
                            nc.scalar.activation(
                                fsl, pso[:], AF.Copy, scale=fscale,
                            )
                    nb = jg
                    for ob in range(NCB):
                        ps2 = psmm2.tile([128, 512], f32, tag="ps2")
                        ics = range(NCB, NIC) if w1fg is not None else range(NIC)
                        for i, ic in enumerate(ics):
                            nc.tensor.matmul(
                                ps2[:],
                                wtt[:, ic, ts(ob, 128)],
                                cats[ic][:, ts(nb, 512)],
                                start=(i == 0),
                                stop=(ic == NIC - 1),
                            )
                        osl = outsb[:, nb, ob, :]
                        if w1fg is not None:
                            # out = (W2@final + bias) + W1@fg in one stt
                            nc.vector.scalar_tensor_tensor(
                                osl, ps2[:], b2t[:, ob:ob + 1],
                                w1fg[:, nb, ob, :],
                                op0=ALU.add, op1=ALU.add,
                            )
                        elif ob == 0 and OB0 == "dve":
                            nc.vector.tensor_scalar_add(
                                osl, ps2[:], b2t[:, ob:ob + 1]
                            )
                        else:
                            nc.scalar.activation(
                                osl, ps2[:], AF.Identity,
                                bias=b2t[:, ob:ob + 1],
                            )
                if os.environ.get("TRN_OSPL", "0") == "1":
                    # per-sample writeback: overlaps under the idle post-A
                    # DMA engine, leaving only sample 3's 1.5us as the tail
                    nc.sync.dma_start(out_d[:, b:b + 1], outsb_all[:, b:b + 1])
                elif b == SPC - 2:
                    nc.sync.dma_start(out_d[:, :SPC - 1], outsb_all[:, :SPC - 1])
                elif b == SPC - 1:
                    nc.sync.dma_start(out_d[:, SPC - 1:], outsb_all[:, SPC - 1:])

            # constants + all-sample fg land first (sample 0's fg split off so
            # its prep chain starts immediately); the A stream follows.
            nc.sync.dma_start(cmt[:], cm_d[:])
            if os.environ.get("TRN_DMAORD", "fgfirst") == "a0early":
                prep_A(0)
                nc.sync.dma_start(fgsb[:], fg_d[:])
                prep_A(1)
                prep_A(2)
                prep_A(3)
            else:
                nc.sync.dma_start(fgsb[:], fg_d[:])
                prep_A(0)
                prep_A(1)
                prep_A(2)
                prep_A(3)
            if True:
                # token schedule: Pb=prep, Kb=K-quant, Wb=W1@fg, Mb=mm1,
                # Fb=fin

                orders = {
                    # best of a ~1000-iter TimelineSim hill-climb over token
                    # interleavings (see ord_search.py)
                    "searched": "P0 P1 K0 W3 K1 P2 W2 P3 M0 K2 F0 M1 K3 F1 "
                                "M2 F2 M3 F3",
                    "hyb2": "P0 P1 P2 P3 K0 K1 M0 F0 K2 M1 F1 K3 M2 M3 F2 F3",
                    "hyb3": "P0 P1 K0 P2 P3 K1 M0 K2 M1 F0 K3 M2 F1 M3 F2 F3",
                    "hyb1": "P0 P1 P2 P3 K0 K1 K2 K3 M0 W1 M1 F0 M2 F1 M3 F2 F3",
                    "nw1": "P0 P1 P2 P3 K0 K1 K2 K3 M0 M1 F0 M2 F1 M3 F2 F3",
                }
                ordstr = os.environ.get("TRN_ORD") or orders.get(
                    SCHED, orders["hyb2"]
                )
                fns = {"P": prep, "K": kq_stage, "W": w1_stage,
                       "M": mm1_stage, "F": fin_stage}
                for tok in ordstr.split():
                    fns[tok[0]](int(tok[1:]))
    nc.compile()
    return nc


def _prep_inputs(foreground, mask, attention_scores, comb_w, comb_b):
    import ml_dtypes

    f32 = np.float32
    f16 = np.float16
    fdat = f16 if DT16 else f32
    # fg: [BS, 128(p), NCB, HW] partition-major, its own tensor
    fg = np.asarray(foreground, dtype=fdat).reshape(BS, NCB, 128, HW)
    fg = np.ascontiguousarray(fg.transpose(0, 2, 1, 3))   # [BS, 128, NCB, HW]
    atf = np.asarray(attention_scores, dtype=f32).reshape(BS, HW, HW)
    m_pre = np.asarray(mask, dtype=f32).reshape(BS, 1, HW)
    if MM1 != "f16":
        # fold the blend coefficient into A: columns scaled by (1-m_p)*32
        atf = atf * ((1.0 - m_pre) * 32.0)
    if MM1 == "dr2":
        a8 = atf.astype(ml_dtypes.float8_e4m3)
        ar8 = (atf - a8.astype(f32)).astype(ml_dtypes.float8_e4m3)
        at = np.stack([ar8, a8], axis=2)        # [BS, HW(krow), 2, HW]
        # -> [BS, p, jhalf, kchunk(8), which, 512pix]: one run per partition,
        # pixel-half-outer so the first half of a sample's DMA already covers
        # output blocks j=0..3 for every k chunk
        at = at.reshape(BS, 8, 128, 2, 2, 512).transpose(0, 2, 4, 1, 3, 5)
        abody = np.ascontiguousarray(at).reshape(BS, 128, 8 * 2 * HW)
    elif MM1 == "dr1":
        at = atf.astype(ml_dtypes.float8_e4m3)
        at = at.reshape(BS, 8, 128, 2, 512).transpose(0, 2, 3, 1, 4)
        abody = np.ascontiguousarray(at).reshape(BS, 128, 8 * HW)
    else:
        at = np.asarray(atf, dtype=fdat)
        at = at.reshape(BS, 8, 128, 2, 512).transpose(0, 2, 3, 1, 4)
        abody = np.ascontiguousarray(at).reshape(
            BS, 128, 8 * HW * 2).view(np.uint8)
    m = np.asarray(mask, dtype=f32).reshape(BS, HW)
    mt = m.reshape(BS, NJ, 128).transpose(0, 2, 1)   # [BS, 128, NJ]
    wt = np.asarray(comb_w, dtype=fdat).T          # [512, 256]
    wtr = wt.reshape(NIC, 128, CH).transpose(1, 0, 2).reshape(128, NIC * CH)
    cst = np.ascontiguousarray(wtr.astype(fdat))
    ident = np.eye(128, dtype=fdat)
    b2 = np.ascontiguousarray(
        np.asarray(comb_b, dtype=f32).reshape(NCB, 128).T
    )
    in_maps = []
    for c in range(NCORES):
        sl = slice(c * SPC, (c + 1) * SPC)
        # merged const block: ident | comb_w^T | b2(bitcast f16) | mask
        mcore = np.ascontiguousarray(
            mt[sl].transpose(1, 0, 2).reshape(128, SPC * NJ)
        )
        fgc = fg[sl]                      # [SPC, 128, NCB, HW]
        # sample 0's fg rides behind the constants in the cm tensor
        fg0cols = fgc[0].reshape(128, NCB * HW).view(f16)
        cm = np.concatenate(
            [ident, cst, b2.view(f16), mcore.view(f16), fg0cols], axis=1
        )
        in_maps.append({
            "attn": np.ascontiguousarray(abody[sl]),
            "fgd": np.ascontiguousarray(fgc[1:].transpose(1, 0, 2, 3)),
            "cm": np.ascontiguousarray(cm),
        })
    return in_maps


def _unshard_out(arr):
    """[ncores*128, SPC, 2, NCB, 512] partition-major layout -> [N, CH, H, W]."""
    arr = np.asarray(arr, dtype=np.float32)
    nc_ = arr.shape[0] // 128
    return (arr.reshape(nc_, 128, SPC, 2, NCB, 512)
            .transpose(0, 2, 4, 1, 3, 5)
            .reshape(nc_ * SPC, CH, H, W))

def run(inputs, trace=False):
    from concourse.bass_utils import run_bass_kernel_spmd

    if "nc" not in _cache:
        _cache["nc"] = _build()
    nc = _cache["nc"]
    in_maps = _prep_inputs(**inputs)
    res = run_bass_kernel_spmd(
        nc, in_maps, core_ids=list(range(NCORES)), trace=trace,
    )
    outs = [_unshard_out(r["out"]) for r in res.results]
    full = np.concatenate(outs, axis=0)
    return full, res


def kernel(**inputs) -> np.ndarray:
    out, _ = run(inputs, trace=False)
    return out


def bench(inputs, iters=20):
    """Build the sharded PJRT executable once, keep inputs device-resident,
    and time repeated executions (min over iters). Mirrors
    bass2jax.run_bass_via_pjrt's multi-core path without per-call retracing."""
    import time

    import jax
    import concourse.mybir as mybir
    from concourse.bass2jax import (
        _bass_exec_p,
        install_neuronx_cc_hook,
        partition_id_tensor,
        Mesh,
        PartitionSpec,
        shard_map,
    )

    install_neuronx_cc_hook()
    if "nc" not in _cache:
        _cache["nc"] = _build()
    nc = _cache["nc"]
    in_maps = _prep_inputs(**inputs)

    partition_name = (
        nc.partition_id_tensor.name if nc.partition_id_tensor else None
    )
    in_names, out_names, out_avals, zero_outs = [], [], [], []
    for alloc in nc.m.functions[0].allocations:
        if not isinstance(alloc, mybir.MemoryLocationSet):
            continue
        name = alloc.memorylocations[0].name
        if alloc.kind == "ExternalInput":
            if name != partition_name:
                in_names.append(name)
        elif alloc.kind == "ExternalOutput":
            out_names.append(name)
            shape = tuple(alloc.tensor_shape)
            dtype = mybir.dt.np(alloc.dtype)
            out_avals.append(jax.core.ShapedArray(shape, dtype))
            zero_outs.append(np.zeros(shape, dtype))
    n_params = len(in_names)
    all_in_names = in_names + out_names
    if partition_name is not None:
        all_in_names = all_in_names + [partition_name]

    def _body(*args):
        operands = list(args)
        if partition_name is not None:
            operands.append(partition_id_tensor())
        outs = _bass_exec_p.bind(
            *operands,
            out_avals=tuple(out_avals),
            in_names=tuple(all_in_names),
            out_names=tuple(out_names),
            lowering_input_output_aliases=(),
            sim_require_finite=True,
            sim_require_nnan=True,
            nc=nc,
        )
        return tuple(outs)

    devices = jax.devices()[:NCORES]
    mesh = Mesh(np.asarray(devices), ("core",))
    in_specs = (PartitionSpec("core"),) * (n_params + len(out_names))
    out_specs = (PartitionSpec("core"),) * len(out_names)
    sharded = jax.jit(
        shard_map(_body, mesh=mesh, in_specs=in_specs, out_specs=out_specs,
                  check_rep=False),
        keep_unused=True,
    )
    concat_in = [
        np.concatenate([in_maps[c][nm] for c in range(NCORES)], axis=0)
        for nm in in_names
    ]
    concat_zeros = [
        np.zeros((NCORES * z.shape[0], *z.shape[1:]), z.dtype) for z in zero_outs
    ]
    sharding = jax.sharding.NamedSharding(mesh, PartitionSpec("core"))
    dev_in = [jax.device_put(x, sharding) for x in concat_in]
    dev_zero = [jax.device_put(x, sharding) for x in concat_zeros]

    # warmup (compiles)
    out = sharded(*dev_in, *dev_zero)
    jax.block_until_ready(out)
    times = []
    for _ in range(iters):
        t0 = time.perf_counter()
        out = sharded(*dev_in, *dev_zero)
        jax.block_until_ready(out)
        times.append(time.perf_counter() - t0)
    full = _unshard_out(out[0])
    return full, times


def bench_chain(inputs, n_chain=64, iters=8):
    """Time N chained NEFF executions inside one dispatch; the slope
    (T_chain - T_single)/(n_chain-1) removes the ~40-80ms axon RPC overhead."""
    import time

    import jax
    import concourse.mybir as mybir
    from concourse.bass2jax import (
        _bass_exec_p,
        install_neuronx_cc_hook,
        partition_id_tensor,
        Mesh,
        PartitionSpec,
        shard_map,
    )

    install_neuronx_cc_hook()
    if "nc" not in _cache:
        _cache["nc"] = _build()
    nc = _cache["nc"]
    in_maps = _prep_inputs(**inputs)

    partition_name = (
        nc.partition_id_tensor.name if nc.partition_id_tensor else None
    )
    in_names, out_names, out_avals, zero_outs = [], [], [], []
    for alloc in nc.m.functions[0].allocations:
        if not isinstance(alloc, mybir.MemoryLocationSet):
            continue
        name = alloc.memorylocations[0].name
        if alloc.kind == "ExternalInput":
            if name != partition_name:
                in_names.append(name)
        elif alloc.kind == "ExternalOutput":
            out_names.append(name)
            shape = tuple(alloc.tensor_shape)
            dtype = mybir.dt.np(alloc.dtype)
            out_avals.append(jax.core.ShapedArray(shape, dtype))
            zero_outs.append(np.zeros(shape, dtype))
    n_params = len(in_names)
    all_in_names = in_names + out_names
    if partition_name is not None:
        all_in_names = all_in_names + [partition_name]

    def _body(*args):
        operands = list(args)
        if partition_name is not None:
            operands.append(partition_id_tensor())
        return tuple(_bass_exec_p.bind(
            *operands,
            out_avals=tuple(out_avals),
            in_names=tuple(all_in_names),
            out_names=tuple(out_names),
            lowering_input_output_aliases=(),
            sim_require_finite=True,
            sim_require_nnan=True,
            nc=nc,
        ))

    devices = jax.devices()[:NCORES]
    mesh = Mesh(np.asarray(devices), ("core",))
    in_specs = (PartitionSpec("core"),) * (n_params + len(out_names))
    out_specs = (PartitionSpec("core"),) * len(out_names)
    sharded = jax.jit(
        shard_map(_body, mesh=mesh, in_specs=in_specs,
                  out_specs=out_specs, check_rep=False),
        keep_unused=True,
    )

    concat_in = [
        np.concatenate([in_maps[c][nm] for c in range(NCORES)], axis=0)
        for nm in in_names
    ]
    concat_zeros = [
        np.zeros((NCORES * z.shape[0], *z.shape[1:]), z.dtype) for z in zero_outs
    ]
    sharding = jax.sharding.NamedSharding(mesh, PartitionSpec("core"))
    dev_in = [jax.device_put(x, sharding) for x in concat_in]
    dev_zero = [jax.device_put(x, sharding) for x in concat_zeros]

    def run_n(n):
        # async-dispatch n executions, chained through the donated output
        # buffers so they serialize on-device; block once at the end
        outs = dev_zero
        for _ in range(n):
            outs = list(sharded(*dev_in, *outs))
        jax.block_until_ready(outs)
        return outs

    out = run_n(1)  # warmup / compile

    def timed_once(n):
        t0 = time.perf_counter()
        run_n(n)
        return time.perf_counter() - t0

    # The axon-tunneled device sees interference windows from other tenants
    # plus per-call RPC jitter; interference only ever adds time, so
    # interleave repeated timings of both chain lengths across spaced-out
    # rounds and take per-endpoint minima. Deep chains (~128) give a 50ms+
    # slope signal that dwarfs the +-5ms RPC jitter.
    n1 = max(2, n_chain // 16)
    t1, tn = 1e18, 1e18
    rounds = max(8, iters // 2)
    for r in range(rounds):
        if r:
            time.sleep(0.7)
        for _ in range(2):
            t1 = min(t1, timed_once(n1))
        # the long chain is the volatile endpoint (its floor swings with the
        # dispatch-pipeline phase) — weight the sampling toward it
        for _ in range(4):
            tn = min(tn, timed_once(n_chain))
    out = run_n(1)
    per_exec = (tn - t1) / (n_chain - n1)
    full = _unshard_out(out[0])
    return full, per_exec, (t1, tn)
